# revision 1
# baseline (speedup 1.0000x reference)
"""Trainium2 Bass kernel for nn_CAWN2 (CAWN-style GNN message passing).

Reference computation (per full input):
  seq = GRUCell(ngh_feat, hidden)                      # [B*2048, 128]
  grouped 2-head attention: q from src, k/v from seq,
  64 neighbors per (b, s) group, additive -1e10 mask,
  softmax, out proj, residual + LayerNorm, 2-layer MLP  -> [B, 32, 128]

Strategy: data-parallel over batch across 8 NeuronCores (32 batches/core).
Per core, a feature-major pipeline processes 16 "supertiles" of 4096
neighbor rows (2 batches):
  - PE transposes x into [d, n] tiles; GRU gates via f32r matmuls
  - fast path (hidden==0, gru biases==0): seq = sigmoid(-gi_z)*tanh(gi_n)
  - w_ks is folded into q, so k is never materialized
  - scores for 8/16-group blocks computed as full cross attention within a
    1024-row chunk; a host-precomputed additive mask (block-diagonal
    structure + input mask, bf16) restores grouping; softmax without
    max-subtraction (|scores| is tiny); exp's accum_out gives row sums
  - attn @ v via PE transposes of attn + accumulated matmuls
  - fc with per-head masked weights fixes the head-diagonal structure
  - residual + LayerNorm via matmul-with-ones partition reductions and a
    Newton rsqrt (avoids the ACT sqrt table set), then the merge MLP
ACT table sets: supertiles are processed in groups of 4: first all GRU
phases (sigmoid set), then all attention phases (exp set).
"""

import os
import sys
from contextlib import ExitStack

import numpy as np

sys.path.insert(0, "/opt/trn_rl_repo")

import ml_dtypes  # noqa: E402

import concourse.bass as bass  # noqa: E402
import concourse.bacc as bacc  # noqa: E402
import concourse.mybir as mybir  # noqa: E402
import concourse.tile as tile  # noqa: E402

F32 = mybir.dt.float32
F32R = mybir.dt.float32r
BF16 = mybir.dt.bfloat16
I32 = mybir.dt.int32
AF = mybir.ActivationFunctionType
ALU = mybir.AluOpType
AX = mybir.AxisListType

N_CORES = 8
B, N_SRC, N_NGH, D, H = 256, 32, 2048, 128, 2
DK = D // H
NN = N_NGH // N_SRC  # 64 neighbors per group
NEG_INF = -1e10
LN_EPS = 1e-5
TEMP = float(np.sqrt(DK))  # 8.0

B_CORE = B // N_CORES          # 32 batches per core
ROWS = B_CORE * N_NGH          # 65536 neighbor rows per core
ST_ROWS = 4096                 # supertile = 2 batches
N_ST = ROWS // ST_ROWS         # 16
N_GC = 8                       # 512-row GRU chunks per supertile
N_AC = 4                       # 1024-row attention chunks per supertile
ST_GROUP = 4                   # supertiles per ACT-table phase group
G_ST = 64                      # (b, s) groups per supertile

_PROG_CACHE: dict = {}


def _mm(ap, use_f32r=True):
    return ap


def build_program(general: bool, use_f32r: bool = True, xt_copy: str = "act",
                  bench_iters: int = 1, small_io: bool = False,
                  tune: dict | None = None):
    tn = {"seq": 5, "stl": 5, "chunk": 3, "att": 2, "sm": 3,
          "xt": 1, "gi": 2, "v": 1, "sc": 1, "psm": 2}
    tn.update(tune or {})
    """Build the per-core SPMD Bass program. Returns (nc, input_names)."""
    nc = bacc.Bacc("TRN2")
    MMDT = F32R if use_f32r else F32

    # ---- DRAM I/O ----
    ngh_rows = ST_ROWS if small_io else ROWS
    t_ngh = nc.dram_tensor("ngh", [ngh_rows, D], F32, kind="ExternalInput")
    t_src = nc.dram_tensor("srcf", [B_CORE * N_SRC, D], F32, kind="ExternalInput")
    n_mask = 1 if small_io else N_ST
    t_mask = nc.dram_tensor("maskfull", [n_mask, 128, 1024], BF16, kind="ExternalInput")
    t_eye = nc.dram_tensor("eye", [128, 128], F32, kind="ExternalInput")
    wnames = ["wqT", "wks", "wihzT", "wihnT", "wvsT",
              "fcw0T", "fcw1T", "m1aT", "m1bT", "m2T"]
    if general:
        wnames += ["wihrT", "whhrT", "whhzT", "whhnT"]
    t_w = {n: nc.dram_tensor(n, [128, 128], MMDT, kind="ExternalInput") for n in wnames}
    t_onesc = nc.dram_tensor("ones2", [128, 2], MMDT, kind="ExternalInput")
    t_onesr = nc.dram_tensor("onesrow", [1, 128], MMDT, kind="ExternalInput")
    t_zeros = nc.dram_tensor("zeros128", [128, 128], MMDT, kind="ExternalInput")
    vnames = ["fc_b", "ln_g", "ln_b", "m1b", "m2b"]
    if general:
        vnames += ["b_r", "b_z", "b_in", "b_hn"]  # b_r=bihr+bhhr, b_z=bihz+bhhz
    t_v = {n: nc.dram_tensor(n, [128, 1], F32, kind="ExternalInput") for n in vnames}
    if general:
        t_hid = nc.dram_tensor("hid", [ROWS, D], F32, kind="ExternalInput")
    t_out = nc.dram_tensor("z", [B_CORE * N_SRC, D], F32, kind="ExternalOutput")

    n_st_io = 1 if small_io else N_ST
    ngh_v = t_ngh[:, :].rearrange("(st gc t p) d -> st gc p t d", st=n_st_io, gc=N_GC, t=4, p=128)
    if general:
        hid_v = t_hid[:, :].rearrange("(st gc t p) d -> st gc p t d", st=N_ST, gc=N_GC, t=4, p=128)

    with tile.TileContext(nc) as tc, ExitStack() as ctx:
        consts = ctx.enter_context(tc.tile_pool(name="consts", bufs=1))
        # big long-lived per-supertile tiles
        p_seq = ctx.enter_context(tc.tile_pool(name="p_seq", bufs=tn["seq"]))
        p_stl = ctx.enter_context(tc.tile_pool(name="p_stl", bufs=tn["stl"]))   # small per-st, long-lived
        p_chunk = ctx.enter_context(tc.tile_pool(name="p_chunk", bufs=tn["chunk"]))
        p_att = ctx.enter_context(tc.tile_pool(name="p_att", bufs=tn["att"]))
        p_sm = ctx.enter_context(tc.tile_pool(name="p_sm", bufs=tn["sm"]))     # small short-lived
        # PSUM pools (8 banks total; budget exactly)
        ps_xt = ctx.enter_context(tc.tile_pool(name="ps_xt", bufs=tn["xt"], space="PSUM"))
        ps_gi = ctx.enter_context(tc.tile_pool(name="ps_gi", bufs=tn["gi"], space="PSUM"))
        ps_v = ctx.enter_context(tc.tile_pool(name="ps_v", bufs=tn["v"], space="PSUM"))
        ps_sc = ctx.enter_context(tc.tile_pool(name="ps_sc", bufs=tn["sc"], space="PSUM"))
        ps_at = ctx.enter_context(tc.tile_pool(name="ps_at", bufs=1, space="PSUM"))
        ps_sm = ctx.enter_context(tc.tile_pool(name="ps_sm", bufs=tn["psm"], space="PSUM"))

        # ---- load constants ----
        eye = consts.tile([128, 128], F32)
        nc.sync.dma_start(out=eye, in_=t_eye[:, :])
        w_sb = {}
        for n in wnames:
            w_sb[n] = consts.tile([128, 128], MMDT, name=f"w_{n}")
            nc.sync.dma_start(out=w_sb[n], in_=t_w[n][:, :])
        v_sb = {}
        for n in vnames:
            v_sb[n] = consts.tile([128, 1], F32, name=f"v_{n}")
            nc.sync.dma_start(out=v_sb[n], in_=t_v[n][:, :])
        ones2 = consts.tile([128, 2], MMDT)
        nc.sync.dma_start(out=ones2, in_=t_onesc[:, :])
        zconst = consts.tile([128, 128], MMDT)
        nc.sync.dma_start(out=zconst, in_=t_zeros[:, :])
        ones_row = consts.tile([1, 128], MMDT)
        nc.sync.dma_start(out=ones_row, in_=t_onesr[:, :])

        def transpose(out_ap, in_ap, base=0, k=128):
            tp = (base, 0) if base else None
            nc.tensor.transpose(out_ap, in_ap, eye[base:base + k, base:base + k],
                                tile_position=tp)

        def gru_phase(st):
            """GRU for one supertile; writes sb_seqT (and returns per-st tiles)."""
            # src rows for this supertile -> srcT, q, q'
            sb_src = p_sm.tile([64, 128], F32, tag="src")
            nc.sync.dma_start(out=sb_src, in_=t_src[st * 64:(st + 1) * 64, :])
            pt_srcT = ps_sm.tile([128, 128], F32, tag="psmall", name="pt_srcT")
            transpose(pt_srcT[:, 0:64], sb_src, k=64)
            sb_srcT = p_stl.tile([128, 64], MMDT, tag="srcT")
            nc.scalar.copy(out=sb_srcT, in_=pt_srcT[:, 0:64])

            pt_q = ps_sm.tile([128, 128], F32, tag="psmall", name="pt_q")
            nc.tensor.matmul(pt_q[:, 0:64], w_sb["wqT"],
                             sb_srcT, start=True, stop=True)
            sb_qT = p_sm.tile([128, 64], F32, tag="qT")
            nc.vector.tensor_copy(out=sb_qT, in_=pt_q[:, 0:64])
            # embed q per (chunk, group, head): col = 32*ac + 2*g + h
            sb_qemb = p_sm.tile([128, 128], MMDT, tag="qemb")
            nc.sync.dma_start(out=sb_qemb, in_=t_zeros[:, :])
            qe_v = sb_qemb[:, :].rearrange("p (cg h) -> p h cg", h=2)
            nc.vector.tensor_copy(out=qe_v[0:64, 0, :], in_=sb_qT[0:64, :])
            nc.vector.tensor_copy(out=qe_v[64:128, 1, :], in_=sb_qT[64:128, :])
            # fold w_ks: q' = w_ks.T @ qemb
            pt_qp = ps_sm.tile([128, 128], F32, tag="psmall", name="pt_qp")
            nc.tensor.matmul(pt_qp, w_sb["wks"],
                             sb_qemb, start=True, stop=True)
            sb_qp = []
            for c in range(N_AC):
                qz = p_stl.tile([128, 128], MMDT, tag="qpz", bufs=8, name=f"qz{c}")
                nc.sync.dma_start(out=qz, in_=zconst)
                nc.vector.tensor_copy(out=qz[:, 32 * c:32 * c + 32],
                                      in_=pt_qp[:, 32 * c:32 * c + 32])
                sb_qp.append(qz)

            sb_seqT = p_seq.tile([128, ST_ROWS], MMDT, tag="seqT")
            for gc in range(N_GC):
                x_rm = p_chunk.tile([128, 4, 128], F32, tag="x_rm")
                nc.sync.dma_start(out=x_rm, in_=ngh_v[0 if small_io else st, gc])
                pt_xt = ps_xt.tile([128, 512], F32, tag="xt")
                for t in range(4):
                    transpose(pt_xt[:, t * 128:(t + 1) * 128], x_rm[:, t, :])
                sb_xT = p_chunk.tile([128, 512], MMDT, tag="xT")
                if xt_copy == "act":
                    nc.scalar.copy(out=sb_xT, in_=pt_xt)
                else:
                    nc.vector.tensor_copy(out=sb_xT, in_=pt_xt)
                if general:
                    h_rm = p_chunk.tile([128, 4, 128], F32, tag="h_rm")
                    nc.sync.dma_start(out=h_rm, in_=hid_v[st, gc])
                    pt_ht = ps_xt.tile([128, 512], F32, tag="xt", name="pt_ht")
                    for t in range(4):
                        transpose(pt_ht[:, t * 128:(t + 1) * 128], h_rm[:, t, :])
                    sb_hT = p_chunk.tile([128, 512], MMDT, tag="hT")
                    nc.scalar.copy(out=sb_hT, in_=pt_ht)

                seq_sl = sb_seqT[:, gc * 512:(gc + 1) * 512]
                if not general:
                    # fast: seq = sigmoid(-gi_z) * tanh(gi_n)
                    pt_gz = ps_gi.tile([128, 512], F32, tag="gi", name="pt_gz")
                    nc.tensor.matmul(pt_gz, w_sb["wihzT"],
                                     sb_xT, start=True, stop=True)
                    pt_gn = ps_gi.tile([128, 512], F32, tag="gi", name="pt_gn")
                    nc.tensor.matmul(pt_gn, w_sb["wihnT"],
                                     sb_xT, start=True, stop=True)
                    sb_zc = p_chunk.tile([128, 512], F32, tag="zc")
                    nc.scalar.activation(out=sb_zc, in_=pt_gz, func=AF.Sigmoid, scale=-1.0)
                    sb_nn = p_chunk.tile([128, 512], F32, tag="nn")
                    nc.scalar.activation(out=sb_nn, in_=pt_gn, func=AF.Tanh)
                    nc.vector.tensor_mul(seq_sl, sb_zc, sb_nn)
                else:
                    # r/z gates: gi + gh accumulated in PSUM
                    pt_gr = ps_gi.tile([128, 512], F32, tag="gi", name="pt_gr")
                    nc.tensor.matmul(pt_gr, w_sb["wihrT"],
                                     sb_xT, start=True, stop=False)
                    nc.tensor.matmul(pt_gr, w_sb["whhrT"],
                                     sb_hT, start=False, stop=True)
                    pt_gz = ps_gi.tile([128, 512], F32, tag="gi", name="pt_gz")
                    nc.tensor.matmul(pt_gz, w_sb["wihzT"],
                                     sb_xT, start=True, stop=False)
                    nc.tensor.matmul(pt_gz, w_sb["whhzT"],
                                     sb_hT, start=False, stop=True)
                    pt_gni = ps_gi.tile([128, 512], F32, tag="gi", name="pt_gni")
                    nc.tensor.matmul(pt_gni, w_sb["wihnT"],
                                     sb_xT, start=True, stop=True)
                    pt_gnh = ps_gi.tile([128, 512], F32, tag="gi", name="pt_gnh")
                    nc.tensor.matmul(pt_gnh, w_sb["whhnT"],
                                     sb_hT, start=True, stop=True)
                    sb_r = p_chunk.tile([128, 512], F32, tag="zc", name="sb_r")
                    nc.scalar.activation(out=sb_r, in_=pt_gr, func=AF.Sigmoid,
                                         bias=v_sb["b_r"])
                    sb_z = p_chunk.tile([128, 512], F32, tag="zc", name="sb_z")
                    nc.scalar.activation(out=sb_z, in_=pt_gz, func=AF.Sigmoid,
                                         bias=v_sb["b_z"])
                    # npre = gi_n + r*(gh_n + b_hn);   (b_hn bcast via tensor_scalar)
                    sb_hnb = p_chunk.tile([128, 512], F32, tag="nn", name="sb_hnb")
                    nc.vector.tensor_scalar_add(sb_hnb, pt_gnh, v_sb["b_hn"])
                    sb_rn = p_chunk.tile([128, 512], F32, tag="nn", name="sb_rn")
                    nc.vector.tensor_mul(sb_rn, sb_r, sb_hnb)
                    sb_np = p_chunk.tile([128, 512], F32, tag="nn", name="sb_np")
                    nc.vector.tensor_add(sb_np, pt_gni, sb_rn)
                    sb_nn = p_chunk.tile([128, 512], F32, tag="nn", name="sb_nn")
                    nc.scalar.activation(out=sb_nn, in_=sb_np, func=AF.Tanh,
                                         bias=v_sb["b_in"])
                    # seq = nn + z*(h - nn)
                    sb_hmn = p_chunk.tile([128, 512], F32, tag="nn", name="sb_hmn")
                    nc.vector.tensor_sub(sb_hmn, sb_hT, sb_nn)
                    sb_zh = p_chunk.tile([128, 512], F32, tag="nn", name="sb_zh")
                    nc.vector.tensor_mul(sb_zh, sb_z, sb_hmn)
                    nc.vector.tensor_add(seq_sl, sb_nn, sb_zh)
            return sb_srcT, sb_qp, sb_seqT

        def attn_phase(st, sb_srcT, sb_qp, sb_seqT):
            # ---- scores per 1024-row chunk ----
            sb_mask = p_att.tile([128, 1024], BF16, tag="mask")
            nc.sync.dma_start(out=sb_mask, in_=t_mask[0 if small_io else st])

            pt_sc = ps_sc.tile([128, 1024], F32, tag="sc")
            for ac in range(N_AC):
                base = ac * 1024
                for half in range(2):
                    nc.tensor.matmul(
                        pt_sc[:, half * 512:(half + 1) * 512],
                        sb_qp[ac],
                        sb_seqT[:, base + half * 512:base + (half + 1) * 512],
                        start=(ac == 0), stop=(ac == N_AC - 1))

            # ---- mask + exp + sums ----
            sb_scm = p_att.tile([128, 1024], F32, tag="scm")
            nc.vector.tensor_add(sb_scm, pt_sc, sb_mask)
            sb_attn = p_att.tile([128, 1024], F32, tag="attn")
            sb_sums = p_sm.tile([128, 1], F32, tag="sums")
            nc.scalar.activation(out=sb_attn, in_=sb_scm, func=AF.Exp,
                                 accum_out=sb_sums)
            sb_rec = p_stl.tile([128, 1], F32, tag="rec")
            sb_sum2 = p_sm.tile([128, 1], F32, tag="sums", name="sb_sum2")
            nc.vector.tensor_scalar_add(sb_sum2, sb_sums, 1e-30)
            nc.vector.reciprocal(sb_rec, sb_sum2)
            sb_attn_n = p_att.tile([128, 1024], F32, tag="attn_n")
            nc.vector.tensor_scalar_mul(sb_attn_n, sb_attn, sb_rec)

            # ---- v (row-major) and attn @ v, per chunk ----
            pt_oaT = ps_sm.tile([128, 128], F32, tag="psmall", name="pt_oaT")
            for ac in range(N_AC):
                base = ac * 1024
                sb_vrm = p_att.tile([128, 8, 128], MMDT, tag="vrm", bufs=2)
                for half in range(2):
                    pt_v = ps_v.tile([128, 512], F32, tag="v")
                    for j in range(4):
                        sl = sb_seqT[:, base + half * 512 + j * 128:
                                     base + half * 512 + (j + 1) * 128]
                        nc.tensor.matmul(pt_v[:, j * 128:(j + 1) * 128],
                                         sl, w_sb["wvsT"],
                                         start=True, stop=True)
                    nc.vector.tensor_copy(out=sb_vrm[:, half * 4:(half + 1) * 4, :],
                                          in_=pt_v)
                pt_at = ps_v.tile([128, 256], F32, tag="v", name="pt_at")
                for j in range(8):
                    transpose(pt_at[:, j * 32:(j + 1) * 32],
                              sb_attn_n[32 * ac:32 * ac + 32, j * 128:(j + 1) * 128],
                              base=32 * ac, k=32)
                sb_at = p_sm.tile([128, 256], MMDT, tag="at")
                nc.vector.tensor_copy(out=sb_at, in_=pt_at)
                pt_oa = ps_sm.tile([32, 128], F32, tag="psmall", name="pt_oa")
                for j in range(8):
                    nc.tensor.matmul(pt_oa,
                                     sb_at[:, j * 32:(j + 1) * 32],
                                     sb_vrm[:, j, :],
                                     start=(j == 0), stop=(j == 7))
                sb_oa = p_sm.tile([32, 128], F32, tag="oa")
                nc.vector.tensor_copy(out=sb_oa, in_=pt_oa)
                transpose(pt_oaT[:, 32 * ac:32 * ac + 32], sb_oa, k=32)
            sb_oaT = p_sm.tile([128, 128], MMDT, tag="oaT")
            nc.vector.tensor_copy(out=sb_oaT, in_=pt_oaT)

            # ---- fc with per-head masked weights ----
            oaT_v = sb_oaT[:, :].rearrange("p (cg h) -> p h cg", h=2)
            pt_fc = ps_sm.tile([128, 128], F32, tag="psmall", name="pt_fc")
            nc.tensor.matmul(pt_fc[:, 0:64], w_sb["fcw0T"],
                             oaT_v[:, 0, :], start=True, stop=False)
            nc.tensor.matmul(pt_fc[:, 0:64], w_sb["fcw1T"],
                             oaT_v[:, 1, :], start=False, stop=True)

            # ---- residual + LayerNorm (feature-major) ----
            sb_x1 = p_sm.tile([128, 64], F32, tag="x1")
            nc.vector.tensor_scalar_add(sb_x1, pt_fc[:, 0:64], v_sb["fc_b"])
            sb_x2 = p_sm.tile([128, 64], MMDT, tag="x2")
            nc.vector.tensor_add(sb_x2, sb_x1, sb_srcT)
            sb_sq = p_sm.tile([128, 64], MMDT, tag="sq")
            nc.scalar.activation(out=sb_sq, in_=sb_x2[:, :].bitcast(F32), func=AF.Square)
            pt_ln = ps_sm.tile([128, 128], F32, tag="psmall", name="pt_ln")
            nc.tensor.matmul(pt_ln[0:2, 0:64], ones2,
                             sb_x2, start=True, stop=True)
            nc.tensor.matmul(pt_ln[0:2, 64:128], ones2,
                             sb_sq, start=True, stop=True)
            sb_stats = p_sm.tile([1, 128], MMDT, tag="ln_stats")
            sb_mu = sb_stats[0:1, 0:64]
            nc.vector.tensor_scalar_mul(sb_mu, pt_ln[0:1, 0:64], 1.0 / 128.0)
            sb_ve = p_sm.tile([1, 64], F32, tag="ln_ve")
            # var + eps = E[x^2] - mu^2 + eps  = (pt_ln[64:]/128 - mu*mu) + eps
            sb_ex2 = p_sm.tile([1, 64], F32, tag="ln_ex2")
            nc.vector.tensor_scalar(sb_ex2, pt_ln[0:1, 64:128], 1.0 / 128.0, LN_EPS,
                                    op0=ALU.mult, op1=ALU.add)
            sb_musq = p_sm.tile([1, 64], F32, tag="ln_musq")
            nc.vector.tensor_mul(sb_musq, sb_mu, sb_mu)
            nc.vector.tensor_sub(sb_ve, sb_ex2, sb_musq)
            # Newton rsqrt of sb_ve
            sb_y = p_sm.tile([1, 64], F32, tag="ln_y")
            sb_yi = p_sm.tile([1, 64], I32, tag="ln_yi")
            nc.vector.tensor_scalar(sb_yi, sb_ve[:, :].bitcast(I32), 1, None,
                                    op0=ALU.arith_shift_right)
            nc.vector.tensor_scalar(sb_y[:, :].bitcast(I32), sb_yi, -1, 0x5F3759DF,
                                    op0=ALU.mult, op1=ALU.add)
            for it in range(3):
                sb_t = p_sm.tile([1, 64], F32, tag="ln_t")
                nc.vector.tensor_mul(sb_t, sb_y, sb_y)
                sb_t2 = p_sm.tile([1, 64], F32, tag="ln_t2")
                nc.vector.tensor_mul(sb_t2, sb_t, sb_ve)
                sb_t3 = p_sm.tile([1, 64], F32, tag="ln_t3")
                nc.vector.tensor_scalar(sb_t3, sb_t2, -0.5, 1.5, op0=ALU.mult, op1=ALU.add)
                if it < 2:
                    sb_y2 = p_sm.tile([1, 64], F32, tag="ln_y2")
                else:
                    sb_y2 = sb_stats[0:1, 64:128]
                nc.vector.tensor_mul(sb_y2, sb_y, sb_t3)
                sb_y = sb_y2
            # broadcast [mu | rstd] across partitions with a K=1 ones matmul
            pt_bc = ps_sm.tile([128, 128], F32, tag="psmall", name="pt_bc")
            nc.tensor.matmul(pt_bc, ones_row, sb_stats,
                             start=True, stop=True)
            sb_xc = p_sm.tile([128, 64], F32, tag="xc")
            nc.vector.tensor_sub(sb_xc, sb_x2, pt_bc[:, 0:64])
            sb_xn0 = p_sm.tile([128, 64], F32, tag="xn0")
            nc.vector.tensor_mul(sb_xn0, sb_xc, pt_bc[:, 64:128])
            sb_xn = p_sm.tile([128, 64], MMDT, tag="xn")
            nc.vector.tensor_scalar(sb_xn, sb_xn0, v_sb["ln_g"], v_sb["ln_b"],
                                    op0=ALU.mult, op1=ALU.add)

            # ---- merge MLP ----
            pt_h1 = ps_sm.tile([128, 128], F32, tag="psmall", name="pt_h1")
            nc.tensor.matmul(pt_h1[:, 0:64], w_sb["m1aT"],
                             sb_xn, start=True, stop=False)
            nc.tensor.matmul(pt_h1[:, 0:64], w_sb["m1bT"],
                             sb_srcT, start=False, stop=True)
            sb_h1 = p_sm.tile([128, 64], MMDT, tag="h1")
            nc.scalar.activation(out=sb_h1, in_=pt_h1[:, 0:64], func=AF.Relu,
                                 bias=v_sb["m1b"])
            pt_z = ps_sm.tile([128, 128], F32, tag="psmall", name="pt_z")
            nc.tensor.matmul(pt_z[:, 0:64], w_sb["m2T"],
                             sb_h1, start=True, stop=True)
            sb_zb = p_sm.tile([128, 64], F32, tag="zb")
            nc.vector.tensor_scalar_add(sb_zb, pt_z[:, 0:64], v_sb["m2b"])
            pt_zr = ps_sm.tile([128, 128], F32, tag="psmall", name="pt_zr")
            transpose(pt_zr[0:64, :], sb_zb)
            sb_zout = p_sm.tile([64, 128], F32, tag="zout")
            nc.scalar.copy(out=sb_zout, in_=pt_zr[0:64, :])
            nc.sync.dma_start(out=t_out[st * 64:(st + 1) * 64, :], in_=sb_zout)

        for _bench in range(bench_iters):
            for grp in range(N_ST // ST_GROUP):
                sts = range(grp * ST_GROUP, (grp + 1) * ST_GROUP)
                carry = [gru_phase(st) for st in sts]
                for st, c in zip(sts, carry):
                    attn_phase(st, *c)

    nc.finalize()
    names = ["ngh", "srcf", "maskfull", "eye"] + wnames + vnames
    if general:
        names.append("hid")
    return nc, names


# ----------------------------------------------------------------------------
# Host side
# ----------------------------------------------------------------------------

def _prep_inputs(inputs, general):
    """Build per-core input maps (numpy) from full-size inputs."""
    f32 = np.float32
    src = np.ascontiguousarray(np.asarray(inputs["src"], f32))
    ngh = np.ascontiguousarray(np.asarray(inputs["ngh_feat"], f32))
    mask = np.asarray(inputs["mask"]).astype(bool)
    w_qs = np.asarray(inputs["w_qs"], f32)
    w_ks = np.asarray(inputs["w_ks"], f32)
    w_vs = np.asarray(inputs["w_vs"], f32)
    fc_w = np.asarray(inputs["fc_w"], f32)
    w_ih = np.asarray(inputs["gru_w_ih"], f32)
    m_fc1 = np.asarray(inputs["m_fc1_w"], f32)
    m_fc2 = np.asarray(inputs["m_fc2_w"], f32)

    com = {
        "eye": np.eye(128, dtype=f32),
        "ones2": np.concatenate([np.ones((128, 1), f32), np.zeros((128, 1), f32)], 1),
        "onesrow": np.ones((1, 128), f32),
        "zeros128": np.zeros((128, 128), f32),
        "wqT": np.ascontiguousarray((w_qs / TEMP).T),
        "wks": np.ascontiguousarray(w_ks),
        "wihzT": np.ascontiguousarray(w_ih[128:256].T),
        "wihnT": np.ascontiguousarray(w_ih[256:384].T),
        "wvsT": np.ascontiguousarray(w_vs.T),
        "fcw0T": np.ascontiguousarray(fc_w.T * (np.arange(128) < 64)[:, None].astype(f32)),
        "fcw1T": np.ascontiguousarray(fc_w.T * (np.arange(128) >= 64)[:, None].astype(f32)),
        "m1aT": np.ascontiguousarray(m_fc1[:, :128].T),
        "m1bT": np.ascontiguousarray(m_fc1[:, 128:].T),
        "m2T": np.ascontiguousarray(m_fc2.T),
        "fc_b": np.asarray(inputs["fc_b"], f32).reshape(128, 1),
        "ln_g": np.asarray(inputs["ln_g"], f32).reshape(128, 1),
        "ln_b": np.asarray(inputs["ln_b"], f32).reshape(128, 1),
        "m1b": np.asarray(inputs["m_fc1_b"], f32).reshape(128, 1),
        "m2b": np.asarray(inputs["m_fc2_b"], f32).reshape(128, 1),
    }
    if general:
        w_hh = np.asarray(inputs["gru_w_hh"], f32)
        b_ih = np.asarray(inputs["gru_b_ih"], f32)
        b_hh = np.asarray(inputs["gru_b_hh"], f32)
        com.update({
            "wihrT": np.ascontiguousarray(w_ih[0:128].T),
            "whhrT": np.ascontiguousarray(w_hh[0:128].T),
            "whhzT": np.ascontiguousarray(w_hh[128:256].T),
            "whhnT": np.ascontiguousarray(w_hh[256:384].T),
            "b_r": (b_ih[0:128] + b_hh[0:128]).reshape(128, 1).astype(f32),
            "b_z": (b_ih[128:256] + b_hh[128:256]).reshape(128, 1).astype(f32),
            "b_in": b_ih[256:384].reshape(128, 1).astype(f32),
            "b_hn": b_hh[256:384].reshape(128, 1).astype(f32),
        })

    # additive mask, per core: [N_ST, 128(=32ac+2g+h), 1024] (bf16)
    m3 = mask.reshape(N_CORES, B_CORE, N_SRC, NN)  # [core, b, s, n]
    st_i = np.arange(N_ST)
    cc_i = np.arange(4)
    g_i = np.arange(16)
    b_idx = 2 * st_i[:, None] + cc_i[None, :] // 2          # [st, cc]
    s_idx = (cc_i[:, None] % 2) * 16 + g_i[None, :]         # [cc, g]
    maskfull_cores = []
    for core in range(N_CORES):
        msel = m3[core][b_idx[:, :, None], s_idx[None, :, :]]   # [st, cc, g, 64]
        vals = np.where(msel, f32(NEG_INF), f32(0.0))           # [st, cc, g, 64]
        out = np.full((N_ST, 4, 16, 2, 16, 64), NEG_INF, f32)
        out[:, :, g_i, :, g_i, :] = vals.transpose(2, 0, 1, 3)[:, :, :, None, :]
        maskfull_cores.append(out.reshape(N_ST, 128, 1024).astype(ml_dtypes.bfloat16))

    in_maps = []
    hid = None
    if general:
        hid = np.ascontiguousarray(np.asarray(inputs["hidden"], f32))
    for core in range(N_CORES):
        m = dict(com)
        m["ngh"] = ngh[core * ROWS:(core + 1) * ROWS]
        m["srcf"] = src[core * B_CORE:(core + 1) * B_CORE].reshape(B_CORE * N_SRC, D)
        m["maskfull"] = maskfull_cores[core]
        if general:
            m["hid"] = hid[core * ROWS:(core + 1) * ROWS]
        in_maps.append(m)
    return in_maps


def _get_program(general, use_f32r=True, xt_copy="act"):
    key = (general, use_f32r, xt_copy)
    if key not in _PROG_CACHE:
        _PROG_CACHE[key] = build_program(general, use_f32r, xt_copy)
    return _PROG_CACHE[key]


def _is_fast_path(inputs):
    if np.asarray(inputs["gru_b_ih"]).any() or np.asarray(inputs["gru_b_hh"]).any():
        return False
    return not np.asarray(inputs["hidden"]).any()


def run(inputs, trace=False, use_f32r=None, xt_copy=None, force_general=None):
    if use_f32r is None:
        use_f32r = os.environ.get("K_F32R", "1") == "1"
    if xt_copy is None:
        xt_copy = os.environ.get("K_XTCOPY", "act")
    from concourse.bass_utils import run_bass_kernel_spmd
    general = (not _is_fast_path(inputs)) if force_general is None else force_general
    nc, _ = _get_program(general, use_f32r, xt_copy)
    in_maps = _prep_inputs(inputs, general)
    res = run_bass_kernel_spmd(nc, in_maps, list(range(N_CORES)), trace=trace)
    z = np.stack([r["z"] for r in res.results], axis=0)  # [8, 1024, 128]
    out = z.reshape(N_CORES, B_CORE, N_SRC, D).reshape(B, N_SRC, D).astype(np.float32)
    return out, res


def kernel(**inputs) -> np.ndarray:
    out, _ = run(inputs, trace=False)
    return out



# revision 5
# speedup vs baseline: 2.5912x; 2.5912x over previous
"""Trainium2 Bass kernel for nn_CAWN2 (CAWN-style GNN message passing).

Data-parallel over batch across 8 NeuronCores (32 batches/core).
Fast path (hidden==0, gru biases==0): see build_fast_program below.
General path: previous-generation program (build_program), kept as fallback.
"""

import os
import sys
from contextlib import ExitStack

import numpy as np

sys.path.insert(0, "/opt/trn_rl_repo")

import ml_dtypes  # noqa: E402

import concourse.bass as bass  # noqa: E402,F401
import concourse.bacc as bacc  # noqa: E402
import concourse.mybir as mybir  # noqa: E402
import concourse.tile as tile  # noqa: E402

F32 = mybir.dt.float32
F32R = mybir.dt.float32r
BF16 = mybir.dt.bfloat16
I32 = mybir.dt.int32
AF = mybir.ActivationFunctionType
ALU = mybir.AluOpType
AX = mybir.AxisListType

N_CORES = 8
B, N_SRC, N_NGH, D, H = 256, 32, 2048, 128, 2
DK = D // H
NN = N_NGH // N_SRC
NEG_INF = -1e10
LN_EPS = 1e-5
TEMP = float(np.sqrt(DK))

B_CORE = B // N_CORES
ROWS = B_CORE * N_NGH
ST_ROWS = 4096
N_ST = ROWS // ST_ROWS
N_GC = 8
N_AC = 4
ST_GROUP = 4
G_ST = 64
TOK = B_CORE * N_SRC

WNAMES = ["wqT", "wks", "wihzT", "wihnT", "wvsT",
          "fcw0T", "fcw1T", "m1aT", "m1bT", "m2T"]
VNAMES = ["fc_b", "ln_g", "ln_b", "m1b", "m2b"]

_PROG_CACHE: dict = {}


def build_fast_program(stt_engine="vector", oacopy="act"):
    nc = bacc.Bacc("TRN2")

    t_xT = nc.dram_tensor("xT", [128, ROWS], BF16, kind="ExternalInput")
    t_srcT = nc.dram_tensor("srcT", [128, TOK], F32, kind="ExternalInput")
    t_srcTb = nc.dram_tensor("srcTb", [128, TOK], BF16, kind="ExternalInput")
    t_maskT = nc.dram_tensor("maskT", [N_ST, 128, 1024], BF16, kind="ExternalInput")
    t_eye = nc.dram_tensor("eye", [128, 128], F32, kind="ExternalInput")
    t_w = {n: nc.dram_tensor(n, [128, 128], BF16, kind="ExternalInput")
           for n in WNAMES}
    t_ones2 = nc.dram_tensor("ones2", [128, 2], F32R, kind="ExternalInput")
    t_onesrow = nc.dram_tensor("onesrow", [1, 128], F32R, kind="ExternalInput")
    t_v = {n: nc.dram_tensor(n, [128, 1], F32, kind="ExternalInput")
           for n in VNAMES}
    t_out = nc.dram_tensor("z", [TOK, D], F32, kind="ExternalOutput")

    with tile.TileContext(nc) as tc, ExitStack() as ctx:
        consts = ctx.enter_context(tc.tile_pool(name="consts", bufs=1))
        persist = ctx.enter_context(tc.tile_pool(name="persist", bufs=1))
        p_x = ctx.enter_context(tc.tile_pool(name="p_x", bufs=3))
        p_seq = ctx.enter_context(tc.tile_pool(name="p_seq", bufs=2))
        p_gru = ctx.enter_context(tc.tile_pool(name="p_gru", bufs=3))
        p_att = ctx.enter_context(tc.tile_pool(name="p_att", bufs=2))
        p_vrm = ctx.enter_context(tc.tile_pool(name="p_vrm", bufs=8))
        p_sm = ctx.enter_context(tc.tile_pool(name="p_sm", bufs=3))
        p_post = ctx.enter_context(tc.tile_pool(name="p_post", bufs=2))
        # PSUM: gi 3 + sT 2 + v 2 + oa 1 = 8 banks
        ps_gi = ctx.enter_context(tc.tile_pool(name="ps_gi", bufs=3, space="PSUM"))
        ps_sT = ctx.enter_context(tc.tile_pool(name="ps_sT", bufs=1, space="PSUM"))
        ps_v = ctx.enter_context(tc.tile_pool(name="ps_v", bufs=2, space="PSUM"))
        ps_oa = ctx.enter_context(tc.tile_pool(name="ps_oa", bufs=1, space="PSUM"))

        # ---- constants ----
        eye = consts.tile([128, 128], F32)
        nc.sync.dma_start(out=eye, in_=t_eye[:, :])
        w_sb = {}
        for n in WNAMES:
            w_sb[n] = consts.tile([128, 128], BF16, name=f"w_{n}")
            nc.sync.dma_start(out=w_sb[n], in_=t_w[n][:, :])
        ones2 = consts.tile([128, 2], F32R)
        nc.sync.dma_start(out=ones2, in_=t_ones2[:, :])
        onesrow = consts.tile([1, 128], F32R)
        nc.sync.dma_start(out=onesrow, in_=t_onesrow[:, :])
        v_sb = {}
        for n in VNAMES:
            v_sb[n] = consts.tile([128, 1], F32, name=f"v_{n}")
            nc.sync.dma_start(out=v_sb[n], in_=t_v[n][:, :])
        srcT = persist.tile([128, TOK], F32, name="srcT_sb")
        nc.sync.dma_start(out=srcT, in_=t_srcT[:, :])
        srcTb = persist.tile([128, TOK], BF16, name="srcTb_sb")
        nc.sync.dma_start(out=srcTb, in_=t_srcTb[:, :])

        stt = nc.gpsimd if stt_engine == "gpsimd" else nc.vector

        # ---- q / qp prep (all supertiles up front) ----
        pt_q = ps_sT.tile([128, 1024], F32, tag="sT", name="pt_q")
        for hlf in range(2):
            nc.tensor.matmul(pt_q[:, hlf * 512:(hlf + 1) * 512], w_sb["wqT"],
                             srcTb[:, hlf * 512:(hlf + 1) * 512],
                             start=True, stop=True)
        sb_q = persist.tile([128, 1024], BF16, name="sb_q")
        nc.vector.tensor_copy(out=sb_q, in_=pt_q)
        qemb = persist.tile([128, 2048], BF16, name="sb_qemb")
        nc.vector.memset(qemb, 0.0)
        qe = qemb[:, :].rearrange("p (st j h) -> p st j h", st=N_ST, j=64, h=2)
        for st in range(N_ST):
            nc.vector.tensor_copy(out=qe[0:64, st, :, 0],
                                  in_=sb_q[0:64, st * 64:(st + 1) * 64])
            nc.vector.tensor_copy(out=qe[64:128, st, :, 1],
                                  in_=sb_q[64:128, st * 64:(st + 1) * 64])
        qp = persist.tile([128, 2048], BF16, name="sb_qp")
        for r in range(4):
            pt_qp = ps_v.tile([128, 512], F32, tag="v", name="pt_qp")
            for k in range(4):
                st = 4 * r + k
                nc.tensor.matmul(pt_qp[:, k * 128:(k + 1) * 128], w_sb["wks"],
                                 qemb[:, st * 128:(st + 1) * 128],
                                 start=True, stop=True)
            nc.vector.tensor_copy(out=qp[:, r * 512:(r + 1) * 512], in_=pt_qp)

        oaT_all = persist.tile([128, N_ST * 128], BF16, name="sb_oaT_all")

        # ---- main pipeline ----
        def gru_chunk(sb_xT, sb_seqT, gc):
            xs = sb_xT[:, gc * 512:(gc + 1) * 512]
            pt_gz = ps_gi.tile([128, 512], F32, tag="gi", name="pt_gz")
            nc.tensor.matmul(pt_gz, w_sb["wihzT"], xs, start=True, stop=True)
            pt_gn = ps_gi.tile([128, 512], F32, tag="gi", name="pt_gn")
            nc.tensor.matmul(pt_gn, w_sb["wihnT"], xs, start=True, stop=True)
            sb_t1 = p_gru.tile([128, 512], BF16, tag="t1")
            nc.scalar.activation(out=sb_t1, in_=pt_gz, func=AF.Tanh, scale=-0.5)
            sb_t2 = p_gru.tile([128, 512], BF16, tag="t2")
            nc.scalar.activation(out=sb_t2, in_=pt_gn, func=AF.Tanh)
            # seq' = (t1 + 1) * t2
            stt.scalar_tensor_tensor(
                out=sb_seqT[:, gc * 512:(gc + 1) * 512], in0=sb_t1, scalar=1.0,
                in1=sb_t2, op0=ALU.add, op1=ALU.mult)

        def sv_slices(st, sb_seqT, pt_sT, vrm_tiles, j0):
            """8 slices of 128 neighbor rows: scores^T + v rows."""
            ac = j0 // 8
            if j0 % 8 == 0:
                vt = p_vrm.tile([128, 8, 130], BF16, tag="vrm",
                                name=f"vrm{st}_{ac}")
                nc.vector.memset(vt[:, :, 128:129], 1.0)
                vrm_tiles[ac] = vt
            vt = vrm_tiles[ac]
            for j in range(j0, j0 + 8):
                sl = sb_seqT[:, j * 128:(j + 1) * 128]
                nc.tensor.matmul(
                    pt_sT[:, j * 32:(j + 1) * 32], sl,
                    qp[:, st * 128 + 32 * ac: st * 128 + 32 * ac + 32],
                    start=True, stop=True)
                if j % 4 == 0:
                    pt_v = ps_v.tile([128, 512], F32, tag="v", name="pt_v")
                    vrm_tiles["pt"] = pt_v
                pt_v = vrm_tiles["pt"]
                nc.tensor.matmul(pt_v[:, (j % 4) * 128:((j % 4) + 1) * 128],
                                 sl, w_sb["wvsT"], start=True, stop=True)
                if j % 4 == 3:
                    q = (j % 8) // 4
                    nc.vector.tensor_copy(
                        out=vt[:, q * 4:(q + 1) * 4, 0:128],
                        in_=pt_v[:, :].rearrange("p (t d) -> p t d", t=4))

        def attn_head(st, pt_sT, sb_mask):
            sb_scm = p_att.tile([128, 1024], BF16, tag="scm")
            nc.vector.tensor_add(sb_scm, pt_sT, sb_mask)
            sb_e = p_att.tile([128, 1024], BF16, tag="e")
            nc.scalar.activation(out=sb_e, in_=sb_scm, func=AF.Exp)
            pt_oa = ps_oa.tile([128, 136], F32, tag="oa", name="pt_oa")
            return sb_e, pt_oa

        def eav_ac(sb_e, pt_oa, vrm_tiles, ac):
            vt = vrm_tiles[ac]
            for jj in range(8):
                j = 8 * ac + jj
                nc.tensor.matmul(pt_oa[32 * ac:32 * ac + 32, 0:129],
                                 sb_e[:, j * 32:(j + 1) * 32],
                                 vt[:, jj, 0:129],
                                 start=(jj == 0), stop=(jj == 7),
                                 tile_position=(0, 32 * ac))

        def oa_tail(st, pt_oa):
            sb_den = p_sm.tile([128, 1], F32, tag="den")
            nc.vector.tensor_scalar_add(sb_den, pt_oa[:, 128:129], 1e-30)
            sb_rec = p_sm.tile([128, 1], F32, tag="rec")
            nc.vector.reciprocal(sb_rec, sb_den)
            sb_oa = p_sm.tile([128, 128], F32, tag="oa_sb")
            nc.vector.tensor_scalar_mul(sb_oa, pt_oa[:, 0:128], sb_rec)
            pt_oaT = ps_oa.tile([128, 136], F32, tag="oa", name="pt_oaT")
            nc.tensor.transpose(pt_oaT[:, 0:128], sb_oa, eye)
            dst = oaT_all[:, st * 128:(st + 1) * 128]
            if oacopy == "act":
                nc.scalar.copy(out=dst, in_=pt_oaT[:, 0:128])
            else:
                nc.vector.tensor_copy(out=dst, in_=pt_oaT[:, 0:128])

        prev = None  # (st, sb_e, pt_oa, vrm_tiles) pending e@v+tail
        cur = None   # (st, pt_sT, sb_mask, vrm_tiles) pending exp
        for st in range(N_ST):
            sb_xT = p_x.tile([128, ST_ROWS], BF16, tag="xT", name=f"x{st}")
            nc.sync.dma_start(out=sb_xT,
                              in_=t_xT[:, st * ST_ROWS:(st + 1) * ST_ROWS])
            sb_mask = p_att.tile([128, 1024], BF16, tag="mask", name=f"m{st}")
            nc.sync.dma_start(out=sb_mask, in_=t_maskT[st])
            sb_seqT = p_seq.tile([128, ST_ROWS], BF16, tag="seqT",
                                 name=f"seq{st}")
            if cur is not None:
                pst, ppt, pmask, pvrm = cur
                sb_e, pt_oa = attn_head(pst, ppt, pmask)
                prev = (pst, sb_e, pt_oa, pvrm)
                cur = None
            for gc in range(4):
                gru_chunk(sb_xT, sb_seqT, gc)
                if prev is not None:
                    eav_ac(prev[1], prev[2], prev[3], gc)
            if prev is not None:
                oa_tail(prev[0], prev[2])
                prev = None
            pt_sT = ps_sT.tile([128, 1024], F32, tag="sT", name=f"sT{st}")
            vrm_tiles = {}
            for gc in range(4, 8):
                gru_chunk(sb_xT, sb_seqT, gc)
                sv_slices(st, sb_seqT, pt_sT, vrm_tiles, 8 * (gc - 4))
            cur = (st, pt_sT, sb_mask, vrm_tiles)

        # drain last supertile
        pst, ppt, pmask, pvrm = cur
        sb_e, pt_oa = attn_head(pst, ppt, pmask)
        for ac in range(4):
            eav_ac(sb_e, pt_oa, pvrm, ac)
        oa_tail(pst, pt_oa)

        # ---- post pipeline: fc + residual + LN + MLP, 512-token halves ----
        oaTv = oaT_all[:, :].rearrange("p (st j h) -> p st h j",
                                       st=N_ST, j=64, h=2)
        for hlf in range(2):
            c0 = hlf * 512
            pt_fc = ps_v.tile([128, 512], F32, tag="v", name="pt_fc")
            for k in range(8):
                st = hlf * 8 + k
                nc.tensor.matmul(pt_fc[:, k * 64:(k + 1) * 64],
                                 w_sb["fcw0T"], oaTv[:, st, 0, :],
                                 start=True, stop=False)
                nc.tensor.matmul(pt_fc[:, k * 64:(k + 1) * 64],
                                 w_sb["fcw1T"], oaTv[:, st, 1, :],
                                 start=False, stop=True)
            sb_x2 = p_post.tile([128, 512], F32R, tag="x2")
            nc.vector.scalar_tensor_tensor(
                out=sb_x2, in0=pt_fc, scalar=v_sb["fc_b"],
                in1=srcT[:, c0:c0 + 512], op0=ALU.add, op1=ALU.add)
            sb_sq = p_post.tile([128, 512], F32R, tag="sq")
            nc.scalar.activation(out=sb_sq, in_=sb_x2[:, :].bitcast(F32), func=AF.Square)
            pt_ln = ps_sT.tile([128, 1024], F32, tag="sT", name="pt_ln")
            nc.tensor.matmul(pt_ln[0:2, 0:512], ones2,
                             sb_x2, start=True, stop=True)
            nc.tensor.matmul(pt_ln[0:2, 512:1024], ones2,
                             sb_sq, start=True, stop=True)
            sb_stats = p_post.tile([1, 1024], F32R, tag="stats")
            sb_mu = sb_stats[0:1, 0:512]
            sb_mu_f = sb_mu.bitcast(F32)
            nc.vector.tensor_scalar_mul(sb_mu, pt_ln[0:1, 0:512], 1.0 / 128.0)
            sb_ex2 = p_post.tile([1, 512], F32, tag="ex2")
            nc.vector.tensor_scalar(sb_ex2, pt_ln[0:1, 512:1024], 1.0 / 128.0,
                                    LN_EPS, op0=ALU.mult, op1=ALU.add)
            sb_musq = p_post.tile([1, 512], F32, tag="musq")
            nc.vector.tensor_mul(sb_musq, sb_mu_f, sb_mu_f)
            sb_ve = p_post.tile([1, 512], F32, tag="ve")
            nc.vector.tensor_sub(sb_ve, sb_ex2, sb_musq)
            sb_yi = p_post.tile([1, 512], I32, tag="yi")
            nc.vector.tensor_scalar(sb_yi, sb_ve[:, :].bitcast(I32), 1, None,
                                    op0=ALU.arith_shift_right)
            sb_y = p_post.tile([1, 512], F32, tag="y0")
            nc.vector.tensor_scalar(sb_y[:, :].bitcast(I32), sb_yi, -1,
                                    0x5F3759DF, op0=ALU.mult, op1=ALU.add)
            for it in range(3):
                sb_t = p_post.tile([1, 512], F32, tag=f"nt{it}")
                nc.vector.tensor_mul(sb_t, sb_y, sb_y)
                sb_t2 = p_post.tile([1, 512], F32, tag=f"nt2{it}")
                nc.vector.tensor_mul(sb_t2, sb_t, sb_ve)
                sb_t3 = p_post.tile([1, 512], F32, tag=f"nt3{it}")
                nc.vector.tensor_scalar(sb_t3, sb_t2, -0.5, 1.5,
                                        op0=ALU.mult, op1=ALU.add)
                sb_y2 = sb_stats[0:1, 512:1024] if it == 2 else \
                    p_post.tile([1, 512], F32, tag=f"ny{it}")
                nc.vector.tensor_mul(sb_y2, sb_y, sb_t3)
                sb_y = sb_y2
            pt_bc = ps_v.tile([128, 512], F32, tag="v", name="pt_bc")
            nc.tensor.matmul(pt_bc, onesrow,
                             sb_stats[0:1, 0:512],
                             start=True, stop=True)
            sb_xc = p_post.tile([128, 512], F32, tag="xc")
            nc.vector.tensor_sub(sb_xc, sb_x2[:, :].bitcast(F32), pt_bc)
            pt_bc2 = ps_v.tile([128, 512], F32, tag="v", name="pt_bc2")
            nc.tensor.matmul(pt_bc2, onesrow,
                             sb_stats[0:1, 512:1024],
                             start=True, stop=True)
            sb_xn0 = p_post.tile([128, 512], F32, tag="xn0")
            nc.vector.tensor_mul(sb_xn0, sb_xc, pt_bc2)
            sb_xn = p_post.tile([128, 512], BF16, tag="xn")
            nc.vector.tensor_scalar(sb_xn, sb_xn0, v_sb["ln_g"], v_sb["ln_b"],
                                    op0=ALU.mult, op1=ALU.add)
            pt_h1 = ps_v.tile([128, 512], F32, tag="v", name="pt_h1")
            nc.tensor.matmul(pt_h1, w_sb["m1aT"], sb_xn, start=True, stop=False)
            nc.tensor.matmul(pt_h1, w_sb["m1bT"], srcTb[:, c0:c0 + 512],
                             start=False, stop=True)
            sb_h1 = p_post.tile([128, 512], BF16, tag="h1")
            nc.scalar.activation(out=sb_h1, in_=pt_h1, func=AF.Relu,
                                 bias=v_sb["m1b"])
            pt_z = ps_v.tile([128, 512], F32, tag="v", name="pt_z")
            nc.tensor.matmul(pt_z, w_sb["m2T"], sb_h1, start=True, stop=True)
            sb_zb = p_post.tile([128, 512], F32, tag="zb")
            nc.vector.tensor_scalar_add(sb_zb, pt_z, v_sb["m2b"])
            sb_zout = p_post.tile([128, 4, 128], F32, tag="zout")
            for k in range(4):
                pt_zr = ps_oa.tile([128, 136], F32, tag="oa", name="pt_zr")
                nc.tensor.transpose(pt_zr[:, 0:128],
                                    sb_zb[:, k * 128:(k + 1) * 128], eye)
                nc.scalar.copy(out=sb_zout[:, k, :], in_=pt_zr[:, 0:128])
            out_v = t_out[:, :].rearrange("(g p) d -> p g d", p=128)
            nc.sync.dma_start(out=out_v[:, hlf * 4:(hlf + 1) * 4, :],
                              in_=sb_zout)

    nc.finalize()
    return nc


def prep_fast_inputs(inputs):
    """Per-core input maps for the fast-path program."""
    f32 = np.float32
    bf16 = ml_dtypes.bfloat16
    src = np.asarray(inputs["src"], f32)
    ngh = np.asarray(inputs["ngh_feat"], f32)
    mask = np.asarray(inputs["mask"]).astype(bool)
    w_qs = np.asarray(inputs["w_qs"], f32)
    w_ks = np.asarray(inputs["w_ks"], f32)
    w_vs = np.asarray(inputs["w_vs"], f32)
    fc_w = np.asarray(inputs["fc_w"], f32)
    w_ih = np.asarray(inputs["gru_w_ih"], f32)
    m_fc1 = np.asarray(inputs["m_fc1_w"], f32)
    m_fc2 = np.asarray(inputs["m_fc2_w"], f32)

    com = {
        "eye": np.eye(128, dtype=f32),
        "ones2": np.concatenate([np.ones((128, 1), f32),
                                 np.zeros((128, 1), f32)], 1),
        "onesrow": np.ones((1, 128), f32),
        "wqT": np.ascontiguousarray((w_qs / (2.0 * TEMP)).T).astype(bf16),
        "wks": np.ascontiguousarray(w_ks).astype(bf16),
        "wihzT": np.ascontiguousarray(w_ih[128:256].T).astype(bf16),
        "wihnT": np.ascontiguousarray(w_ih[256:384].T).astype(bf16),
        "wvsT": np.ascontiguousarray(w_vs.T / 2.0).astype(bf16),
        "fcw0T": np.ascontiguousarray(
            fc_w.T * (np.arange(128) < 64)[:, None].astype(f32)).astype(bf16),
        "fcw1T": np.ascontiguousarray(
            fc_w.T * (np.arange(128) >= 64)[:, None].astype(f32)).astype(bf16),
        "m1aT": np.ascontiguousarray(m_fc1[:, :128].T).astype(bf16),
        "m1bT": np.ascontiguousarray(m_fc1[:, 128:].T).astype(bf16),
        "m2T": np.ascontiguousarray(m_fc2.T).astype(bf16),
        "fc_b": np.asarray(inputs["fc_b"], f32).reshape(128, 1),
        "ln_g": np.asarray(inputs["ln_g"], f32).reshape(128, 1),
        "ln_b": np.asarray(inputs["ln_b"], f32).reshape(128, 1),
        "m1b": np.asarray(inputs["m_fc1_b"], f32).reshape(128, 1),
        "m2b": np.asarray(inputs["m_fc2_b"], f32).reshape(128, 1),
    }

    # additive mask, transposed-score layout: [st, p(=n%128), 32*sl + 2g + h]
    st_i = np.arange(N_ST).reshape(-1, 1, 1, 1)
    sl_i = np.arange(32).reshape(1, -1, 1, 1)
    p_i = np.arange(128).reshape(1, 1, -1, 1)
    g_i = np.arange(16).reshape(1, 1, 1, -1)
    n_in_b = (sl_i % 16) * 128 + p_i                 # [1,32,128,1]
    s_r = n_in_b // NN                                # group of the neighbor row
    s_slot = ((sl_i // 8) % 2) * 16 + g_i             # group of the score slot
    b_loc = sl_i // 16                                # batch within supertile

    m3 = mask.reshape(N_CORES, B_CORE, N_NGH)
    in_maps = []
    for core in range(N_CORES):
        mm = m3[core]                                 # [32, 2048] True = drop
        b_glob = (2 * st_i + b_loc)                   # [16,32,1,1]
        dropped = mm[b_glob, n_in_b]                  # [16,32,128,16]
        allow = (s_r == s_slot) & ~dropped            # [16,32,128,16]
        mk = np.where(allow[..., None], f32(0.0), f32(NEG_INF))  # [...,16,2]
        mk = np.broadcast_to(mk, (N_ST, 32, 128, 16, 2))
        mk = mk.transpose(0, 2, 1, 3, 4).reshape(N_ST, 128, 1024)
        m = dict(com)
        m["maskT"] = np.ascontiguousarray(mk).astype(bf16)
        xr = ngh[core * ROWS:(core + 1) * ROWS]
        m["xT"] = np.ascontiguousarray(xr.T.astype(bf16))
        sc = src[core * B_CORE:(core + 1) * B_CORE].reshape(TOK, D)
        m["srcT"] = np.ascontiguousarray(sc.T)
        m["srcTb"] = np.ascontiguousarray(sc.T.astype(bf16))
        in_maps.append(m)
    return in_maps


def build_program(general: bool, use_f32r: bool = True, xt_copy: str = "act",
                  bench_iters: int = 1, small_io: bool = False,
                  tune: dict | None = None):
    tn = {"seq": 5, "stl": 5, "chunk": 3, "att": 2, "sm": 3,
          "xt": 1, "gi": 2, "v": 1, "sc": 1, "psm": 2}
    tn.update(tune or {})
    """Build the per-core SPMD Bass program. Returns (nc, input_names)."""
    nc = bacc.Bacc("TRN2")
    MMDT = F32R if use_f32r else F32

    # ---- DRAM I/O ----
    ngh_rows = ST_ROWS if small_io else ROWS
    t_ngh = nc.dram_tensor("ngh", [ngh_rows, D], F32, kind="ExternalInput")
    t_src = nc.dram_tensor("srcf", [B_CORE * N_SRC, D], F32, kind="ExternalInput")
    n_mask = 1 if small_io else N_ST
    t_mask = nc.dram_tensor("maskfull", [n_mask, 128, 1024], BF16, kind="ExternalInput")
    t_eye = nc.dram_tensor("eye", [128, 128], F32, kind="ExternalInput")
    wnames = ["wqT", "wks", "wihzT", "wihnT", "wvsT",
              "fcw0T", "fcw1T", "m1aT", "m1bT", "m2T"]
    if general:
        wnames += ["wihrT", "whhrT", "whhzT", "whhnT"]
    t_w = {n: nc.dram_tensor(n, [128, 128], MMDT, kind="ExternalInput") for n in wnames}
    t_onesc = nc.dram_tensor("ones2", [128, 2], MMDT, kind="ExternalInput")
    t_onesr = nc.dram_tensor("onesrow", [1, 128], MMDT, kind="ExternalInput")
    t_zeros = nc.dram_tensor("zeros128", [128, 128], MMDT, kind="ExternalInput")
    vnames = ["fc_b", "ln_g", "ln_b", "m1b", "m2b"]
    if general:
        vnames += ["b_r", "b_z", "b_in", "b_hn"]  # b_r=bihr+bhhr, b_z=bihz+bhhz
    t_v = {n: nc.dram_tensor(n, [128, 1], F32, kind="ExternalInput") for n in vnames}
    if general:
        t_hid = nc.dram_tensor("hid", [ROWS, D], F32, kind="ExternalInput")
    t_out = nc.dram_tensor("z", [B_CORE * N_SRC, D], F32, kind="ExternalOutput")

    n_st_io = 1 if small_io else N_ST
    ngh_v = t_ngh[:, :].rearrange("(st gc t p) d -> st gc p t d", st=n_st_io, gc=N_GC, t=4, p=128)
    if general:
        hid_v = t_hid[:, :].rearrange("(st gc t p) d -> st gc p t d", st=N_ST, gc=N_GC, t=4, p=128)

    with tile.TileContext(nc) as tc, ExitStack() as ctx:
        consts = ctx.enter_context(tc.tile_pool(name="consts", bufs=1))
        # big long-lived per-supertile tiles
        p_seq = ctx.enter_context(tc.tile_pool(name="p_seq", bufs=tn["seq"]))
        p_stl = ctx.enter_context(tc.tile_pool(name="p_stl", bufs=tn["stl"]))   # small per-st, long-lived
        p_chunk = ctx.enter_context(tc.tile_pool(name="p_chunk", bufs=tn["chunk"]))
        p_att = ctx.enter_context(tc.tile_pool(name="p_att", bufs=tn["att"]))
        p_sm = ctx.enter_context(tc.tile_pool(name="p_sm", bufs=tn["sm"]))     # small short-lived
        # PSUM pools (8 banks total; budget exactly)
        ps_xt = ctx.enter_context(tc.tile_pool(name="ps_xt", bufs=tn["xt"], space="PSUM"))
        ps_gi = ctx.enter_context(tc.tile_pool(name="ps_gi", bufs=tn["gi"], space="PSUM"))
        ps_v = ctx.enter_context(tc.tile_pool(name="ps_v", bufs=tn["v"], space="PSUM"))
        ps_sc = ctx.enter_context(tc.tile_pool(name="ps_sc", bufs=tn["sc"], space="PSUM"))
        ps_at = ctx.enter_context(tc.tile_pool(name="ps_at", bufs=1, space="PSUM"))
        ps_sm = ctx.enter_context(tc.tile_pool(name="ps_sm", bufs=tn["psm"], space="PSUM"))

        # ---- load constants ----
        eye = consts.tile([128, 128], F32)
        nc.sync.dma_start(out=eye, in_=t_eye[:, :])
        w_sb = {}
        for n in wnames:
            w_sb[n] = consts.tile([128, 128], MMDT, name=f"w_{n}")
            nc.sync.dma_start(out=w_sb[n], in_=t_w[n][:, :])
        v_sb = {}
        for n in vnames:
            v_sb[n] = consts.tile([128, 1], F32, name=f"v_{n}")
            nc.sync.dma_start(out=v_sb[n], in_=t_v[n][:, :])
        ones2 = consts.tile([128, 2], MMDT)
        nc.sync.dma_start(out=ones2, in_=t_onesc[:, :])
        zconst = consts.tile([128, 128], MMDT)
        nc.sync.dma_start(out=zconst, in_=t_zeros[:, :])
        ones_row = consts.tile([1, 128], MMDT)
        nc.sync.dma_start(out=ones_row, in_=t_onesr[:, :])

        def transpose(out_ap, in_ap, base=0, k=128):
            tp = (base, 0) if base else None
            nc.tensor.transpose(out_ap, in_ap, eye[base:base + k, base:base + k],
                                tile_position=tp)

        def gru_phase(st):
            """GRU for one supertile; writes sb_seqT (and returns per-st tiles)."""
            # src rows for this supertile -> srcT, q, q'
            sb_src = p_sm.tile([64, 128], F32, tag="src")
            nc.sync.dma_start(out=sb_src, in_=t_src[st * 64:(st + 1) * 64, :])
            pt_srcT = ps_sm.tile([128, 128], F32, tag="psmall", name="pt_srcT")
            transpose(pt_srcT[:, 0:64], sb_src, k=64)
            sb_srcT = p_stl.tile([128, 64], MMDT, tag="srcT")
            nc.scalar.copy(out=sb_srcT, in_=pt_srcT[:, 0:64])

            pt_q = ps_sm.tile([128, 128], F32, tag="psmall", name="pt_q")
            nc.tensor.matmul(pt_q[:, 0:64], w_sb["wqT"],
                             sb_srcT, start=True, stop=True)
            sb_qT = p_sm.tile([128, 64], F32, tag="qT")
            nc.vector.tensor_copy(out=sb_qT, in_=pt_q[:, 0:64])
            # embed q per (chunk, group, head): col = 32*ac + 2*g + h
            sb_qemb = p_sm.tile([128, 128], MMDT, tag="qemb")
            nc.sync.dma_start(out=sb_qemb, in_=t_zeros[:, :])
            qe_v = sb_qemb[:, :].rearrange("p (cg h) -> p h cg", h=2)
            nc.vector.tensor_copy(out=qe_v[0:64, 0, :], in_=sb_qT[0:64, :])
            nc.vector.tensor_copy(out=qe_v[64:128, 1, :], in_=sb_qT[64:128, :])
            # fold w_ks: q' = w_ks.T @ qemb
            pt_qp = ps_sm.tile([128, 128], F32, tag="psmall", name="pt_qp")
            nc.tensor.matmul(pt_qp, w_sb["wks"],
                             sb_qemb, start=True, stop=True)
            sb_qp = []
            for c in range(N_AC):
                qz = p_stl.tile([128, 128], MMDT, tag="qpz", bufs=8, name=f"qz{c}")
                nc.sync.dma_start(out=qz, in_=zconst)
                nc.vector.tensor_copy(out=qz[:, 32 * c:32 * c + 32],
                                      in_=pt_qp[:, 32 * c:32 * c + 32])
                sb_qp.append(qz)

            sb_seqT = p_seq.tile([128, ST_ROWS], MMDT, tag="seqT")
            for gc in range(N_GC):
                x_rm = p_chunk.tile([128, 4, 128], F32, tag="x_rm")
                nc.sync.dma_start(out=x_rm, in_=ngh_v[0 if small_io else st, gc])
                pt_xt = ps_xt.tile([128, 512], F32, tag="xt")
                for t in range(4):
                    transpose(pt_xt[:, t * 128:(t + 1) * 128], x_rm[:, t, :])
                sb_xT = p_chunk.tile([128, 512], MMDT, tag="xT")
                if xt_copy == "act":
                    nc.scalar.copy(out=sb_xT, in_=pt_xt)
                else:
                    nc.vector.tensor_copy(out=sb_xT, in_=pt_xt)
                if general:
                    h_rm = p_chunk.tile([128, 4, 128], F32, tag="h_rm")
                    nc.sync.dma_start(out=h_rm, in_=hid_v[st, gc])
                    pt_ht = ps_xt.tile([128, 512], F32, tag="xt", name="pt_ht")
                    for t in range(4):
                        transpose(pt_ht[:, t * 128:(t + 1) * 128], h_rm[:, t, :])
                    sb_hT = p_chunk.tile([128, 512], MMDT, tag="hT")
                    nc.scalar.copy(out=sb_hT, in_=pt_ht)

                seq_sl = sb_seqT[:, gc * 512:(gc + 1) * 512]
                if not general:
                    # fast: seq = sigmoid(-gi_z) * tanh(gi_n)
                    pt_gz = ps_gi.tile([128, 512], F32, tag="gi", name="pt_gz")
                    nc.tensor.matmul(pt_gz, w_sb["wihzT"],
                                     sb_xT, start=True, stop=True)
                    pt_gn = ps_gi.tile([128, 512], F32, tag="gi", name="pt_gn")
                    nc.tensor.matmul(pt_gn, w_sb["wihnT"],
                                     sb_xT, start=True, stop=True)
                    sb_zc = p_chunk.tile([128, 512], F32, tag="zc")
                    nc.scalar.activation(out=sb_zc, in_=pt_gz, func=AF.Sigmoid, scale=-1.0)
                    sb_nn = p_chunk.tile([128, 512], F32, tag="nn")
                    nc.scalar.activation(out=sb_nn, in_=pt_gn, func=AF.Tanh)
                    nc.vector.tensor_mul(seq_sl, sb_zc, sb_nn)
                else:
                    # r/z gates: gi + gh accumulated in PSUM
                    pt_gr = ps_gi.tile([128, 512], F32, tag="gi", name="pt_gr")
                    nc.tensor.matmul(pt_gr, w_sb["wihrT"],
                                     sb_xT, start=True, stop=False)
                    nc.tensor.matmul(pt_gr, w_sb["whhrT"],
                                     sb_hT, start=False, stop=True)
                    pt_gz = ps_gi.tile([128, 512], F32, tag="gi", name="pt_gz")
                    nc.tensor.matmul(pt_gz, w_sb["wihzT"],
                                     sb_xT, start=True, stop=False)
                    nc.tensor.matmul(pt_gz, w_sb["whhzT"],
                                     sb_hT, start=False, stop=True)
                    pt_gni = ps_gi.tile([128, 512], F32, tag="gi", name="pt_gni")
                    nc.tensor.matmul(pt_gni, w_sb["wihnT"],
                                     sb_xT, start=True, stop=True)
                    pt_gnh = ps_gi.tile([128, 512], F32, tag="gi", name="pt_gnh")
                    nc.tensor.matmul(pt_gnh, w_sb["whhnT"],
                                     sb_hT, start=True, stop=True)
                    sb_r = p_chunk.tile([128, 512], F32, tag="zc", name="sb_r")
                    nc.scalar.activation(out=sb_r, in_=pt_gr, func=AF.Sigmoid,
                                         bias=v_sb["b_r"])
                    sb_z = p_chunk.tile([128, 512], F32, tag="zc", name="sb_z")
                    nc.scalar.activation(out=sb_z, in_=pt_gz, func=AF.Sigmoid,
                                         bias=v_sb["b_z"])
                    # npre = gi_n + r*(gh_n + b_hn);   (b_hn bcast via tensor_scalar)
                    sb_hnb = p_chunk.tile([128, 512], F32, tag="nn", name="sb_hnb")
                    nc.vector.tensor_scalar_add(sb_hnb, pt_gnh, v_sb["b_hn"])
                    sb_rn = p_chunk.tile([128, 512], F32, tag="nn", name="sb_rn")
                    nc.vector.tensor_mul(sb_rn, sb_r, sb_hnb)
                    sb_np = p_chunk.tile([128, 512], F32, tag="nn", name="sb_np")
                    nc.vector.tensor_add(sb_np, pt_gni, sb_rn)
                    sb_nn = p_chunk.tile([128, 512], F32, tag="nn", name="sb_nn")
                    nc.scalar.activation(out=sb_nn, in_=sb_np, func=AF.Tanh,
                                         bias=v_sb["b_in"])
                    # seq = nn + z*(h - nn)
                    sb_hmn = p_chunk.tile([128, 512], F32, tag="nn", name="sb_hmn")
                    nc.vector.tensor_sub(sb_hmn, sb_hT, sb_nn)
                    sb_zh = p_chunk.tile([128, 512], F32, tag="nn", name="sb_zh")
                    nc.vector.tensor_mul(sb_zh, sb_z, sb_hmn)
                    nc.vector.tensor_add(seq_sl, sb_nn, sb_zh)
            return sb_srcT, sb_qp, sb_seqT

        def attn_phase(st, sb_srcT, sb_qp, sb_seqT):
            # ---- scores per 1024-row chunk ----
            sb_mask = p_att.tile([128, 1024], BF16, tag="mask")
            nc.sync.dma_start(out=sb_mask, in_=t_mask[0 if small_io else st])

            pt_sc = ps_sc.tile([128, 1024], F32, tag="sc")
            for ac in range(N_AC):
                base = ac * 1024
                for half in range(2):
                    nc.tensor.matmul(
                        pt_sc[:, half * 512:(half + 1) * 512],
                        sb_qp[ac],
                        sb_seqT[:, base + half * 512:base + (half + 1) * 512],
                        start=(ac == 0), stop=(ac == N_AC - 1))

            # ---- mask + exp + sums ----
            sb_scm = p_att.tile([128, 1024], F32, tag="scm")
            nc.vector.tensor_add(sb_scm, pt_sc, sb_mask)
            sb_attn = p_att.tile([128, 1024], F32, tag="attn")
            sb_sums = p_sm.tile([128, 1], F32, tag="sums")
            nc.scalar.activation(out=sb_attn, in_=sb_scm, func=AF.Exp,
                                 accum_out=sb_sums)
            sb_rec = p_stl.tile([128, 1], F32, tag="rec")
            sb_sum2 = p_sm.tile([128, 1], F32, tag="sums", name="sb_sum2")
            nc.vector.tensor_scalar_add(sb_sum2, sb_sums, 1e-30)
            nc.vector.reciprocal(sb_rec, sb_sum2)
            sb_attn_n = p_att.tile([128, 1024], F32, tag="attn_n")
            nc.vector.tensor_scalar_mul(sb_attn_n, sb_attn, sb_rec)

            # ---- v (row-major) and attn @ v, per chunk ----
            pt_oaT = ps_sm.tile([128, 128], F32, tag="psmall", name="pt_oaT")
            for ac in range(N_AC):
                base = ac * 1024
                sb_vrm = p_att.tile([128, 8, 128], MMDT, tag="vrm", bufs=2)
                for half in range(2):
                    pt_v = ps_v.tile([128, 512], F32, tag="v")
                    for j in range(4):
                        sl = sb_seqT[:, base + half * 512 + j * 128:
                                     base + half * 512 + (j + 1) * 128]
                        nc.tensor.matmul(pt_v[:, j * 128:(j + 1) * 128],
                                         sl, w_sb["wvsT"],
                                         start=True, stop=True)
                    nc.vector.tensor_copy(out=sb_vrm[:, half * 4:(half + 1) * 4, :],
                                          in_=pt_v)
                pt_at = ps_v.tile([128, 256], F32, tag="v", name="pt_at")
                for j in range(8):
                    transpose(pt_at[:, j * 32:(j + 1) * 32],
                              sb_attn_n[32 * ac:32 * ac + 32, j * 128:(j + 1) * 128],
                              base=32 * ac, k=32)
                sb_at = p_sm.tile([128, 256], MMDT, tag="at")
                nc.vector.tensor_copy(out=sb_at, in_=pt_at)
                pt_oa = ps_sm.tile([32, 128], F32, tag="psmall", name="pt_oa")
                for j in range(8):
                    nc.tensor.matmul(pt_oa,
                                     sb_at[:, j * 32:(j + 1) * 32],
                                     sb_vrm[:, j, :],
                                     start=(j == 0), stop=(j == 7))
                sb_oa = p_sm.tile([32, 128], F32, tag="oa")
                nc.vector.tensor_copy(out=sb_oa, in_=pt_oa)
                transpose(pt_oaT[:, 32 * ac:32 * ac + 32], sb_oa, k=32)
            sb_oaT = p_sm.tile([128, 128], MMDT, tag="oaT")
            nc.vector.tensor_copy(out=sb_oaT, in_=pt_oaT)

            # ---- fc with per-head masked weights ----
            oaT_v = sb_oaT[:, :].rearrange("p (cg h) -> p h cg", h=2)
            pt_fc = ps_sm.tile([128, 128], F32, tag="psmall", name="pt_fc")
            nc.tensor.matmul(pt_fc[:, 0:64], w_sb["fcw0T"],
                             oaT_v[:, 0, :], start=True, stop=False)
            nc.tensor.matmul(pt_fc[:, 0:64], w_sb["fcw1T"],
                             oaT_v[:, 1, :], start=False, stop=True)

            # ---- residual + LayerNorm (feature-major) ----
            sb_x1 = p_sm.tile([128, 64], F32, tag="x1")
            nc.vector.tensor_scalar_add(sb_x1, pt_fc[:, 0:64], v_sb["fc_b"])
            sb_x2 = p_sm.tile([128, 64], MMDT, tag="x2")
            nc.vector.tensor_add(sb_x2, sb_x1, sb_srcT)
            sb_sq = p_sm.tile([128, 64], MMDT, tag="sq")
            nc.scalar.activation(out=sb_sq, in_=sb_x2[:, :].bitcast(F32), func=AF.Square)
            pt_ln = ps_sm.tile([128, 128], F32, tag="psmall", name="pt_ln")
            nc.tensor.matmul(pt_ln[0:2, 0:64], ones2,
                             sb_x2, start=True, stop=True)
            nc.tensor.matmul(pt_ln[0:2, 64:128], ones2,
                             sb_sq, start=True, stop=True)
            sb_stats = p_sm.tile([1, 128], MMDT, tag="ln_stats")
            sb_mu = sb_stats[0:1, 0:64]
            nc.vector.tensor_scalar_mul(sb_mu, pt_ln[0:1, 0:64], 1.0 / 128.0)
            sb_ve = p_sm.tile([1, 64], F32, tag="ln_ve")
            # var + eps = E[x^2] - mu^2 + eps  = (pt_ln[64:]/128 - mu*mu) + eps
            sb_ex2 = p_sm.tile([1, 64], F32, tag="ln_ex2")
            nc.vector.tensor_scalar(sb_ex2, pt_ln[0:1, 64:128], 1.0 / 128.0, LN_EPS,
                                    op0=ALU.mult, op1=ALU.add)
            sb_musq = p_sm.tile([1, 64], F32, tag="ln_musq")
            nc.vector.tensor_mul(sb_musq, sb_mu_f, sb_mu_f)
            nc.vector.tensor_sub(sb_ve, sb_ex2, sb_musq)
            # Newton rsqrt of sb_ve
            sb_y = p_sm.tile([1, 64], F32, tag="ln_y")
            sb_yi = p_sm.tile([1, 64], I32, tag="ln_yi")
            nc.vector.tensor_scalar(sb_yi, sb_ve[:, :].bitcast(I32), 1, None,
                                    op0=ALU.arith_shift_right)
            nc.vector.tensor_scalar(sb_y[:, :].bitcast(I32), sb_yi, -1, 0x5F3759DF,
                                    op0=ALU.mult, op1=ALU.add)
            for it in range(3):
                sb_t = p_sm.tile([1, 64], F32, tag="ln_t")
                nc.vector.tensor_mul(sb_t, sb_y, sb_y)
                sb_t2 = p_sm.tile([1, 64], F32, tag="ln_t2")
                nc.vector.tensor_mul(sb_t2, sb_t, sb_ve)
                sb_t3 = p_sm.tile([1, 64], F32, tag="ln_t3")
                nc.vector.tensor_scalar(sb_t3, sb_t2, -0.5, 1.5, op0=ALU.mult, op1=ALU.add)
                if it < 2:
                    sb_y2 = p_sm.tile([1, 64], F32, tag="ln_y2")
                else:
                    sb_y2 = sb_stats[0:1, 64:128]
                nc.vector.tensor_mul(sb_y2, sb_y, sb_t3)
                sb_y = sb_y2
            # broadcast [mu | rstd] across partitions with a K=1 ones matmul
            pt_bc = ps_sm.tile([128, 128], F32, tag="psmall", name="pt_bc")
            nc.tensor.matmul(pt_bc, ones_row, sb_stats,
                             start=True, stop=True)
            sb_xc = p_sm.tile([128, 64], F32, tag="xc")
            nc.vector.tensor_sub(sb_xc, sb_x2, pt_bc[:, 0:64])
            sb_xn0 = p_sm.tile([128, 64], F32, tag="xn0")
            nc.vector.tensor_mul(sb_xn0, sb_xc, pt_bc[:, 64:128])
            sb_xn = p_sm.tile([128, 64], MMDT, tag="xn")
            nc.vector.tensor_scalar(sb_xn, sb_xn0, v_sb["ln_g"], v_sb["ln_b"],
                                    op0=ALU.mult, op1=ALU.add)

            # ---- merge MLP ----
            pt_h1 = ps_sm.tile([128, 128], F32, tag="psmall", name="pt_h1")
            nc.tensor.matmul(pt_h1[:, 0:64], w_sb["m1aT"],
                             sb_xn, start=True, stop=False)
            nc.tensor.matmul(pt_h1[:, 0:64], w_sb["m1bT"],
                             sb_srcT, start=False, stop=True)
            sb_h1 = p_sm.tile([128, 64], MMDT, tag="h1")
            nc.scalar.activation(out=sb_h1, in_=pt_h1[:, 0:64], func=AF.Relu,
                                 bias=v_sb["m1b"])
            pt_z = ps_sm.tile([128, 128], F32, tag="psmall", name="pt_z")
            nc.tensor.matmul(pt_z[:, 0:64], w_sb["m2T"],
                             sb_h1, start=True, stop=True)
            sb_zb = p_sm.tile([128, 64], F32, tag="zb")
            nc.vector.tensor_scalar_add(sb_zb, pt_z[:, 0:64], v_sb["m2b"])
            pt_zr = ps_sm.tile([128, 128], F32, tag="psmall", name="pt_zr")
            transpose(pt_zr[0:64, :], sb_zb)
            sb_zout = p_sm.tile([64, 128], F32, tag="zout")
            nc.scalar.copy(out=sb_zout, in_=pt_zr[0:64, :])
            nc.sync.dma_start(out=t_out[st * 64:(st + 1) * 64, :], in_=sb_zout)

        for _bench in range(bench_iters):
            for grp in range(N_ST // ST_GROUP):
                sts = range(grp * ST_GROUP, (grp + 1) * ST_GROUP)
                carry = [gru_phase(st) for st in sts]
                for st, c in zip(sts, carry):
                    attn_phase(st, *c)

    nc.finalize()
    names = ["ngh", "srcf", "maskfull", "eye"] + wnames + vnames
    if general:
        names.append("hid")
    return nc, names



def _prep_inputs(inputs, general):
    """Build per-core input maps (numpy) from full-size inputs."""
    f32 = np.float32
    src = np.ascontiguousarray(np.asarray(inputs["src"], f32))
    ngh = np.ascontiguousarray(np.asarray(inputs["ngh_feat"], f32))
    mask = np.asarray(inputs["mask"]).astype(bool)
    w_qs = np.asarray(inputs["w_qs"], f32)
    w_ks = np.asarray(inputs["w_ks"], f32)
    w_vs = np.asarray(inputs["w_vs"], f32)
    fc_w = np.asarray(inputs["fc_w"], f32)
    w_ih = np.asarray(inputs["gru_w_ih"], f32)
    m_fc1 = np.asarray(inputs["m_fc1_w"], f32)
    m_fc2 = np.asarray(inputs["m_fc2_w"], f32)

    com = {
        "eye": np.eye(128, dtype=f32),
        "ones2": np.concatenate([np.ones((128, 1), f32), np.zeros((128, 1), f32)], 1),
        "onesrow": np.ones((1, 128), f32),
        "zeros128": np.zeros((128, 128), f32),
        "wqT": np.ascontiguousarray((w_qs / TEMP).T),
        "wks": np.ascontiguousarray(w_ks),
        "wihzT": np.ascontiguousarray(w_ih[128:256].T),
        "wihnT": np.ascontiguousarray(w_ih[256:384].T),
        "wvsT": np.ascontiguousarray(w_vs.T),
        "fcw0T": np.ascontiguousarray(fc_w.T * (np.arange(128) < 64)[:, None].astype(f32)),
        "fcw1T": np.ascontiguousarray(fc_w.T * (np.arange(128) >= 64)[:, None].astype(f32)),
        "m1aT": np.ascontiguousarray(m_fc1[:, :128].T),
        "m1bT": np.ascontiguousarray(m_fc1[:, 128:].T),
        "m2T": np.ascontiguousarray(m_fc2.T),
        "fc_b": np.asarray(inputs["fc_b"], f32).reshape(128, 1),
        "ln_g": np.asarray(inputs["ln_g"], f32).reshape(128, 1),
        "ln_b": np.asarray(inputs["ln_b"], f32).reshape(128, 1),
        "m1b": np.asarray(inputs["m_fc1_b"], f32).reshape(128, 1),
        "m2b": np.asarray(inputs["m_fc2_b"], f32).reshape(128, 1),
    }
    if general:
        w_hh = np.asarray(inputs["gru_w_hh"], f32)
        b_ih = np.asarray(inputs["gru_b_ih"], f32)
        b_hh = np.asarray(inputs["gru_b_hh"], f32)
        com.update({
            "wihrT": np.ascontiguousarray(w_ih[0:128].T),
            "whhrT": np.ascontiguousarray(w_hh[0:128].T),
            "whhzT": np.ascontiguousarray(w_hh[128:256].T),
            "whhnT": np.ascontiguousarray(w_hh[256:384].T),
            "b_r": (b_ih[0:128] + b_hh[0:128]).reshape(128, 1).astype(f32),
            "b_z": (b_ih[128:256] + b_hh[128:256]).reshape(128, 1).astype(f32),
            "b_in": b_ih[256:384].reshape(128, 1).astype(f32),
            "b_hn": b_hh[256:384].reshape(128, 1).astype(f32),
        })

    # additive mask, per core: [N_ST, 128(=32ac+2g+h), 1024] (bf16)
    m3 = mask.reshape(N_CORES, B_CORE, N_SRC, NN)  # [core, b, s, n]
    st_i = np.arange(N_ST)
    cc_i = np.arange(4)
    g_i = np.arange(16)
    b_idx = 2 * st_i[:, None] + cc_i[None, :] // 2          # [st, cc]
    s_idx = (cc_i[:, None] % 2) * 16 + g_i[None, :]         # [cc, g]
    maskfull_cores = []
    for core in range(N_CORES):
        msel = m3[core][b_idx[:, :, None], s_idx[None, :, :]]   # [st, cc, g, 64]
        vals = np.where(msel, f32(NEG_INF), f32(0.0))           # [st, cc, g, 64]
        out = np.full((N_ST, 4, 16, 2, 16, 64), NEG_INF, f32)
        out[:, :, g_i, :, g_i, :] = vals.transpose(2, 0, 1, 3)[:, :, :, None, :]
        maskfull_cores.append(out.reshape(N_ST, 128, 1024).astype(ml_dtypes.bfloat16))

    in_maps = []
    hid = None
    if general:
        hid = np.ascontiguousarray(np.asarray(inputs["hidden"], f32))
    for core in range(N_CORES):
        m = dict(com)
        m["ngh"] = ngh[core * ROWS:(core + 1) * ROWS]
        m["srcf"] = src[core * B_CORE:(core + 1) * B_CORE].reshape(B_CORE * N_SRC, D)
        m["maskfull"] = maskfull_cores[core]
        if general:
            m["hid"] = hid[core * ROWS:(core + 1) * ROWS]
        in_maps.append(m)
    return in_maps




# ----------------------------------------------------------------------------
# Dispatch
# ----------------------------------------------------------------------------

def _get_fast_program(stt_engine, oacopy):
    key = ("fast", stt_engine, oacopy)
    if key not in _PROG_CACHE:
        _PROG_CACHE[key] = build_fast_program(stt_engine, oacopy)
    return _PROG_CACHE[key]


def _get_general_program():
    key = ("general",)
    if key not in _PROG_CACHE:
        _PROG_CACHE[key] = build_program(True, True, "act")
    return _PROG_CACHE[key]


def _is_fast_path(inputs):
    if np.asarray(inputs["gru_b_ih"]).any() or np.asarray(inputs["gru_b_hh"]).any():
        return False
    return not np.asarray(inputs["hidden"]).any()


def run(inputs, trace=False, force_general=None):
    from concourse.bass_utils import run_bass_kernel_spmd
    general = (not _is_fast_path(inputs)) if force_general is None else force_general
    if general:
        nc, _ = _get_general_program()
        in_maps = _prep_inputs(inputs, True)
    else:
        stt_engine = os.environ.get("K_STT", "vector")
        oacopy = os.environ.get("K_OACOPY", "act")
        nc = _get_fast_program(stt_engine, oacopy)
        in_maps = prep_fast_inputs(inputs)
    res = run_bass_kernel_spmd(nc, in_maps, list(range(N_CORES)), trace=trace)
    z = np.stack([r["z"] for r in res.results], axis=0)
    out = z.reshape(N_CORES, B_CORE, N_SRC, D).reshape(B, N_SRC, D).astype(np.float32)
    return out, res


def kernel(**inputs) -> np.ndarray:
    out, _ = run(inputs, trace=False)
    return out
